# revision 1
# baseline (speedup 1.0000x reference)
"""Trainium2 Bass kernel for nn_EnsembleLayer (MoE one-hot routing).

Reference math (per token i, expert e = expert_idx[i]):
    out[i] = W[e] @ logits[i] + b[e] + W_prior[e] @ prior_logits[i] + b_prior[e]

Strategy:
  * Host-side routing: sort tokens by expert (the reference wastefully
    computes all 16 experts per token; we compute only the routed one).
  * Expert-parallel over 8 NeuronCores: core c owns experts (2c, 2c+1),
    each padded to a static capacity of C tokens.
  * Per expert slot the device computes  V.T @ Z  where
        Z = [X_e.T ; Xp_e.T]          (K=2048, C tokens)
        V = [W[e].T ; W_prior[e].T]   (K=2048, 1024 out)
    i.e. posterior and prior fused into one K=2048 contraction, both
    accumulating in the same PSUM tile.
  * Bias (zero in this problem, but handled anyway) is added on host.
  * Tokens overflowing the static capacity (cannot happen for the fixed
    seed's counts, max 295 < C) fall back to exact numpy on host.

Precision strategy for the matmuls (PE TensorEngine):
  * "fp32r" : single-pass FP32R (1s+8e+11m, 1 cyc/row at N>=256). ~1.5e-4
              max scale-relative error.
  * "bf16x3": bf16 hi/lo split, 3 accumulating passes
              (hi@hi + hi@lo + lo@hi). ~5e-6 max scale-relative error.
  * "fp32"  : native fp32 matmul (4 cyc/row). ~2e-7.
"""

import sys

sys.path.insert(0, "/opt/trn_rl_repo")

import ml_dtypes
import numpy as np

import concourse.mybir as mybir
import concourse.tile as tile
from concourse import bacc
from concourse.bass_utils import run_bass_kernel_spmd

dt = mybir.dt

# ---- problem constants (hardcoded per contract) ----
B = 4096
IN_F = 1024
OUT_F = 1024
E = 16
N_CORES = 8
EXPERTS_PER_CORE = E // N_CORES  # 2
P = 128
K = 2 * IN_F  # 2048: posterior + prior fused contraction
KO = K // P  # 16 k-tiles
MO = OUT_F // P  # 8 m-tiles
C = 320  # static token capacity per expert (seed-0 max count is 295)

STRATEGY = "bf16x3"  # one of: "fp32r", "bf16x3", "fp32"


def _round_fp32r(x: np.ndarray) -> np.ndarray:
    """Round fp32 to the FP32R grid (1s+8e+11m, RN-even)."""
    u = np.ascontiguousarray(x).view(np.uint32)
    r = (u + np.uint32(0x7FF) + ((u >> np.uint32(12)) & np.uint32(1))) & np.uint32(
        0xFFFFF000
    )
    return r.view(np.float32)


def _split_bf16(x: np.ndarray):
    hi = x.astype(ml_dtypes.bfloat16)
    lo = (x - hi.astype(np.float32)).astype(ml_dtypes.bfloat16)
    return hi, lo


def build_nc(strategy: str = STRATEGY):
    """Build the SPMD per-core Bass program.

    Inputs per core (leading dim j = expert slot 0/1):
      zt  [2, P, KO, C]      tokens, k-tiled transposed   (fp32r / fp32)
      vt  [2, MO, P, KO, P]  weights, k&m-tiled           (fp32r / fp32)
      (bf16x3: zhi/zlo and vhi/vlo in bf16 instead)
    Output:
      yt  [2, MO, P, C] fp32   yt[j,m,mi,n] = out-feature m*P+mi of token n
    """
    nc = bacc.Bacc("TRN2", target_bir_lowering=False, debug=False)

    if strategy == "bf16x3":
        zhi = nc.dram_tensor("zhi", [2, P, KO, C], dt.bfloat16, kind="ExternalInput").ap()
        zlo = nc.dram_tensor("zlo", [2, P, KO, C], dt.bfloat16, kind="ExternalInput").ap()
        vhi = nc.dram_tensor("vhi", [2, MO, P, KO, P], dt.bfloat16, kind="ExternalInput").ap()
        vlo = nc.dram_tensor("vlo", [2, MO, P, KO, P], dt.bfloat16, kind="ExternalInput").ap()
    else:
        mm_dt = dt.float32r if strategy == "fp32r" else dt.float32
        zt = nc.dram_tensor("zt", [2, P, KO, C], mm_dt, kind="ExternalInput").ap()
        vt = nc.dram_tensor("vt", [2, MO, P, KO, P], mm_dt, kind="ExternalInput").ap()
    yt = nc.dram_tensor("yt", [2, MO, P, C], dt.float32, kind="ExternalOutput").ap()

    with tile.TileContext(nc) as tc:
        with (
            tc.tile_pool(name="zp", bufs=1) as zp,
            tc.tile_pool(name="wp", bufs=4) as wp,
            tc.tile_pool(name="ps", bufs=4, space="PSUM") as ps,
            tc.tile_pool(name="op", bufs=4) as op,
        ):
            if strategy == "bf16x3":
                z_hi = []
                z_lo = []
                for j in range(2):
                    th = zp.tile([P, KO, C], dt.bfloat16, tag=f"zhi{j}")
                    nc.sync.dma_start(th[:], zhi[j])
                    tl = zp.tile([P, KO, C], dt.bfloat16, tag=f"zlo{j}")
                    nc.sync.dma_start(tl[:], zlo[j])
                    z_hi.append(th)
                    z_lo.append(tl)
            else:
                z_tiles = []
                for j in range(2):
                    t = zp.tile([P, KO, C], zt.dtype, tag=f"z{j}")
                    nc.sync.dma_start(t[:], zt[j])
                    z_tiles.append(t)

            for j in range(2):
                for m in range(MO):
                    if strategy == "bf16x3":
                        wh = wp.tile([P, KO, P], dt.bfloat16, tag="wh")
                        nc.sync.dma_start(wh[:], vhi[j, m])
                        wl = wp.tile([P, KO, P], dt.bfloat16, tag="wl")
                        nc.sync.dma_start(wl[:], vlo[j, m])
                        terms = [
                            (wh, z_hi[j]),
                            (wh, z_lo[j]),
                            (wl, z_hi[j]),
                        ]
                    else:
                        w = wp.tile([P, KO, P], vt.dtype, tag="w")
                        nc.sync.dma_start(w[:], vt[j, m])
                        terms = [(w, z_tiles[j])]

                    pt = ps.tile([P, C], dt.float32, tag="psum")
                    n_mm = len(terms) * KO
                    i = 0
                    for wt_t, zt_t in terms:
                        for k in range(KO):
                            nc.tensor.matmul(
                                pt[:],
                                wt_t[:, k, :],
                                zt_t[:, k, :],
                                start=(i == 0),
                                stop=(i == n_mm - 1),
                            )
                            i += 1
                    o = op.tile([P, C], dt.float32, tag="out")
                    nc.any.tensor_copy(out=o[:], in_=pt[:])
                    nc.sync.dma_start(yt[j, m], o[:])

    nc.compile()
    return nc


_NC_CACHE: dict = {}


def _get_nc(strategy: str):
    if strategy not in _NC_CACHE:
        _NC_CACHE[strategy] = build_nc(strategy)
    return _NC_CACHE[strategy]


def _prepare_in_maps(logits, prior_logits, W, W_prior, expert_idx, strategy):
    """Route tokens and build the 8 per-core input maps.

    Returns (in_maps, routed_idx, overflow) where routed_idx[e] is the array
    of token indices assigned to expert e's capacity slots (in slot order)
    and overflow is the list of (token_idx, expert) that did not fit.
    """
    in_maps = []
    routed_idx = []
    overflow = []
    for e in range(E):
        idx = np.nonzero(expert_idx == e)[0]
        if len(idx) > C:
            overflow.extend((int(i), e) for i in idx[C:])
            idx = idx[:C]
        routed_idx.append(idx)

    for c in range(N_CORES):
        zt_c = np.zeros((2, P, KO, C), np.float32)
        vt_c = np.empty((2, MO, P, KO, P), np.float32)
        for j in range(EXPERTS_PER_CORE):
            e = EXPERTS_PER_CORE * c + j
            idx = routed_idx[e]
            n_e = len(idx)
            # Z = [X_e.T ; Xp_e.T]  -> [K, C] -> tiled [P, KO, C]
            Z = np.zeros((K, C), np.float32)
            Z[:IN_F, :n_e] = logits[idx].T
            Z[IN_F:, :n_e] = prior_logits[idx].T
            zt_c[j] = Z.reshape(KO, P, C).transpose(1, 0, 2)
            # V = [W[e].T ; Wp[e].T] -> [K, OUT_F] -> tiled [MO, P, KO, P]
            V = np.concatenate([W[e].T, W_prior[e].T], axis=0)
            vt_c[j] = V.reshape(KO, P, MO, P).transpose(2, 1, 0, 3)

        if strategy == "bf16x3":
            zhi, zlo = _split_bf16(zt_c)
            vhi, vlo = _split_bf16(vt_c)
            in_maps.append({"zhi": zhi, "zlo": zlo, "vhi": vhi, "vlo": vlo})
        elif strategy == "fp32r":
            in_maps.append({"zt": _round_fp32r(zt_c), "vt": _round_fp32r(vt_c)})
        else:
            in_maps.append({"zt": zt_c, "vt": vt_c})
    return in_maps, routed_idx, overflow


def _gather_output(results, routed_idx, overflow, logits, prior_logits, W, b,
                   W_prior, b_prior, expert_idx):
    out = np.empty((B, OUT_F), np.float32)
    for c in range(N_CORES):
        yt = results[c]["yt"]  # [2, MO, P, C]
        for j in range(EXPERTS_PER_CORE):
            e = EXPERTS_PER_CORE * c + j
            idx = routed_idx[e]
            if len(idx) == 0:
                continue
            y = yt[j].reshape(OUT_F, C)  # [out, C]
            out[idx] = y[:, : len(idx)].T
    # bias (zero in this problem, added for faithfulness)
    bias = b + b_prior  # [E, OUT_F]
    if np.any(bias):
        out += bias[expert_idx]
    for i, e in overflow:
        out[i] = (
            W[e] @ logits[i]
            + b[e]
            + W_prior[e] @ prior_logits[i]
            + b_prior[e]
        )
    return out[:, None, :]


def run(inputs: dict, strategy: str = STRATEGY, trace: bool = False):
    """Run the kernel; returns (output, BassKernelResults)."""
    logits = np.asarray(inputs["logits"], np.float32)
    prior_logits = np.asarray(inputs["prior_logits"], np.float32)
    W = np.asarray(inputs["W"], np.float32)
    b = np.asarray(inputs["b"], np.float32)
    W_prior = np.asarray(inputs["W_prior"], np.float32)
    b_prior = np.asarray(inputs["b_prior"], np.float32)
    expert_idx = np.asarray(inputs["expert_idx"])

    nc = _get_nc(strategy)
    in_maps, routed_idx, overflow = _prepare_in_maps(
        logits, prior_logits, W, W_prior, expert_idx, strategy
    )
    br = run_bass_kernel_spmd(nc, in_maps, list(range(N_CORES)), trace=trace)
    out = _gather_output(
        br.results, routed_idx, overflow, logits, prior_logits, W, b, W_prior,
        b_prior, expert_idx,
    )
    return out, br


def kernel(**inputs) -> np.ndarray:
    out, _ = run(inputs, STRATEGY)
    return out


# revision 2
# speedup vs baseline: 1.7629x; 1.7629x over previous
"""Trainium2 Bass kernel for nn_EnsembleLayer (MoE one-hot routing).

Reference math (per token i, expert e = expert_idx[i]):
    out[i] = W[e] @ logits[i] + b[e] + W_prior[e] @ prior_logits[i] + b_prior[e]

Strategy:
  * Host-side routing: sort tokens by expert (the reference wastefully
    computes all 16 experts per token; we compute only the routed one).
  * Expert-parallel over 8 NeuronCores: core c owns experts (2c, 2c+1),
    each padded to a static capacity of C tokens.
  * Per expert slot the device computes  V.T @ Z  where
        Z = [X_e.T ; Xp_e.T]          (K=2048, C tokens)
        V = [W[e].T ; W_prior[e].T]   (K=2048, 1024 out)
    i.e. posterior and prior fused into one K=2048 contraction, both
    accumulating in the same PSUM tile.
  * Bias (zero in this problem, but handled anyway) is added on host.
  * Tokens overflowing the static capacity (cannot happen for the fixed
    seed's counts, max 295 < C) fall back to exact numpy on host.

Precision strategy for the matmuls (PE TensorEngine):
  * "fp32r" : single-pass FP32R (1s+8e+11m, 1 cyc/row at N>=256). ~1.5e-4
              max scale-relative error.
  * "bf16x3": bf16 hi/lo split, 3 accumulating passes
              (hi@hi + hi@lo + lo@hi). ~5e-6 max scale-relative error.
  * "fp32"  : native fp32 matmul (4 cyc/row). ~2e-7.
"""

import sys

sys.path.insert(0, "/opt/trn_rl_repo")

import ml_dtypes
import numpy as np

import concourse.mybir as mybir
import concourse.tile as tile
from concourse import bacc
from concourse.bass_utils import run_bass_kernel_spmd

dt = mybir.dt

# ---- problem constants (hardcoded per contract) ----
B = 4096
IN_F = 1024
OUT_F = 1024
E = 16
N_CORES = 8
EXPERTS_PER_CORE = E // N_CORES  # 2
P = 128
K = 2 * IN_F  # 2048: posterior + prior fused contraction
KO = K // P  # 16 k-tiles
MO = OUT_F // P  # 8 m-tiles
C = 320  # static token capacity per expert (seed-0 max count is 295)

STRATEGY = "bf16x3"  # one of: "fp32r", "bf16x3", "fp32"


def _round_fp32r(x: np.ndarray) -> np.ndarray:
    """Round fp32 to the FP32R grid (1s+8e+11m, RN-even)."""
    u = np.ascontiguousarray(x).view(np.uint32)
    r = (u + np.uint32(0x7FF) + ((u >> np.uint32(12)) & np.uint32(1))) & np.uint32(
        0xFFFFF000
    )
    return r.view(np.float32)


def _split_bf16(x: np.ndarray):
    hi = x.astype(ml_dtypes.bfloat16)
    lo = (x - hi.astype(np.float32)).astype(ml_dtypes.bfloat16)
    return hi, lo


def build_nc(strategy: str = STRATEGY):
    """Build the SPMD per-core Bass program.

    Inputs per core (leading dim j = expert slot 0/1):
      zt  [2, P, KO, C]      tokens, k-tiled transposed   (fp32r / fp32)
      vt  [2, MO, P, KO, P]  weights, k&m-tiled           (fp32r / fp32)
      (bf16x3: zhi/zlo and vhi/vlo in bf16 instead)
    Output:
      yt  [2, MO, P, C] fp32   yt[j,m,mi,n] = out-feature m*P+mi of token n
    """
    nc = bacc.Bacc("TRN2", target_bir_lowering=False, debug=False)

    if strategy == "bf16x3":
        zhi = nc.dram_tensor("zhi", [2, P, KO, C], dt.bfloat16, kind="ExternalInput").ap()
        zlo = nc.dram_tensor("zlo", [2, P, KO, C], dt.bfloat16, kind="ExternalInput").ap()
        vhi = nc.dram_tensor("vhi", [2, MO, P, KO, P], dt.bfloat16, kind="ExternalInput").ap()
        vlo = nc.dram_tensor("vlo", [2, MO, P, KO, P], dt.bfloat16, kind="ExternalInput").ap()
    else:
        mm_dt = dt.float32r if strategy == "fp32r" else dt.float32
        zt = nc.dram_tensor("zt", [2, P, KO, C], mm_dt, kind="ExternalInput").ap()
        vt = nc.dram_tensor("vt", [2, MO, P, KO, P], mm_dt, kind="ExternalInput").ap()
    yt = nc.dram_tensor("yt", [2, MO, P, C], dt.float32, kind="ExternalOutput").ap()

    with tile.TileContext(nc) as tc:
        with (
            tc.tile_pool(name="zp", bufs=1) as zp,
            tc.tile_pool(name="wp", bufs=4) as wp,
            tc.tile_pool(name="ps", bufs=4, space="PSUM") as ps,
            tc.tile_pool(name="op", bufs=4) as op,
        ):
            if strategy == "bf16x3":
                z_hi = []
                z_lo = []
                for j in range(2):
                    th = zp.tile([P, KO, C], dt.bfloat16, tag=f"zhi{j}")
                    nc.sync.dma_start(th[:], zhi[j])
                    tl = zp.tile([P, KO, C], dt.bfloat16, tag=f"zlo{j}")
                    nc.sync.dma_start(tl[:], zlo[j])
                    z_hi.append(th)
                    z_lo.append(tl)
            else:
                z_tiles = []
                for j in range(2):
                    t = zp.tile([P, KO, C], zt.dtype, tag=f"z{j}")
                    nc.sync.dma_start(t[:], zt[j])
                    z_tiles.append(t)

            for j in range(2):
                for m in range(MO):
                    if strategy == "bf16x3":
                        wh = wp.tile([P, KO, P], dt.bfloat16, tag="wh")
                        nc.sync.dma_start(wh[:], vhi[j, m])
                        wl = wp.tile([P, KO, P], dt.bfloat16, tag="wl")
                        nc.sync.dma_start(wl[:], vlo[j, m])
                        terms = [
                            (wh, z_hi[j]),
                            (wh, z_lo[j]),
                            (wl, z_hi[j]),
                        ]
                    else:
                        w = wp.tile([P, KO, P], vt.dtype, tag="w")
                        nc.sync.dma_start(w[:], vt[j, m])
                        terms = [(w, z_tiles[j])]

                    pt = ps.tile([P, C], dt.float32, tag="psum")
                    n_mm = len(terms) * KO
                    i = 0
                    for wt_t, zt_t in terms:
                        for k in range(KO):
                            nc.tensor.matmul(
                                pt[:],
                                wt_t[:, k, :],
                                zt_t[:, k, :],
                                start=(i == 0),
                                stop=(i == n_mm - 1),
                            )
                            i += 1
                    o = op.tile([P, C], dt.float32, tag="out")
                    # explicit DVE: nc.any routes this to ScalarE (9x slower)
                    nc.vector.tensor_copy(out=o[:], in_=pt[:])
                    nc.sync.dma_start(yt[j, m], o[:])

    nc.compile()
    return nc


_NC_CACHE: dict = {}


def _get_nc(strategy: str):
    if strategy not in _NC_CACHE:
        _NC_CACHE[strategy] = build_nc(strategy)
    return _NC_CACHE[strategy]


def _prepare_in_maps(logits, prior_logits, W, W_prior, expert_idx, strategy):
    """Route tokens and build the 8 per-core input maps.

    Returns (in_maps, routed_idx, overflow) where routed_idx[e] is the array
    of token indices assigned to expert e's capacity slots (in slot order)
    and overflow is the list of (token_idx, expert) that did not fit.
    """
    in_maps = []
    routed_idx = []
    overflow = []
    for e in range(E):
        idx = np.nonzero(expert_idx == e)[0]
        if len(idx) > C:
            overflow.extend((int(i), e) for i in idx[C:])
            idx = idx[:C]
        routed_idx.append(idx)

    for c in range(N_CORES):
        zt_c = np.zeros((2, P, KO, C), np.float32)
        vt_c = np.empty((2, MO, P, KO, P), np.float32)
        for j in range(EXPERTS_PER_CORE):
            e = EXPERTS_PER_CORE * c + j
            idx = routed_idx[e]
            n_e = len(idx)
            # Z = [X_e.T ; Xp_e.T]  -> [K, C] -> tiled [P, KO, C]
            Z = np.zeros((K, C), np.float32)
            Z[:IN_F, :n_e] = logits[idx].T
            Z[IN_F:, :n_e] = prior_logits[idx].T
            zt_c[j] = Z.reshape(KO, P, C).transpose(1, 0, 2)
            # V = [W[e].T ; Wp[e].T] -> [K, OUT_F] -> tiled [MO, P, KO, P]
            V = np.concatenate([W[e].T, W_prior[e].T], axis=0)
            vt_c[j] = V.reshape(KO, P, MO, P).transpose(2, 1, 0, 3)

        if strategy == "bf16x3":
            zhi, zlo = _split_bf16(zt_c)
            vhi, vlo = _split_bf16(vt_c)
            in_maps.append({"zhi": zhi, "zlo": zlo, "vhi": vhi, "vlo": vlo})
        elif strategy == "fp32r":
            in_maps.append({"zt": _round_fp32r(zt_c), "vt": _round_fp32r(vt_c)})
        else:
            in_maps.append({"zt": zt_c, "vt": vt_c})
    return in_maps, routed_idx, overflow


def _gather_output(results, routed_idx, overflow, logits, prior_logits, W, b,
                   W_prior, b_prior, expert_idx):
    out = np.empty((B, OUT_F), np.float32)
    for c in range(N_CORES):
        yt = results[c]["yt"]  # [2, MO, P, C]
        for j in range(EXPERTS_PER_CORE):
            e = EXPERTS_PER_CORE * c + j
            idx = routed_idx[e]
            if len(idx) == 0:
                continue
            y = yt[j].reshape(OUT_F, C)  # [out, C]
            out[idx] = y[:, : len(idx)].T
    # bias (zero in this problem, added for faithfulness)
    bias = b + b_prior  # [E, OUT_F]
    if np.any(bias):
        out += bias[expert_idx]
    for i, e in overflow:
        out[i] = (
            W[e] @ logits[i]
            + b[e]
            + W_prior[e] @ prior_logits[i]
            + b_prior[e]
        )
    return out[:, None, :]


def run(inputs: dict, strategy: str = STRATEGY, trace: bool = False):
    """Run the kernel; returns (output, BassKernelResults)."""
    logits = np.asarray(inputs["logits"], np.float32)
    prior_logits = np.asarray(inputs["prior_logits"], np.float32)
    W = np.asarray(inputs["W"], np.float32)
    b = np.asarray(inputs["b"], np.float32)
    W_prior = np.asarray(inputs["W_prior"], np.float32)
    b_prior = np.asarray(inputs["b_prior"], np.float32)
    expert_idx = np.asarray(inputs["expert_idx"])

    nc = _get_nc(strategy)
    in_maps, routed_idx, overflow = _prepare_in_maps(
        logits, prior_logits, W, W_prior, expert_idx, strategy
    )
    br = run_bass_kernel_spmd(nc, in_maps, list(range(N_CORES)), trace=trace)
    out = _gather_output(
        br.results, routed_idx, overflow, logits, prior_logits, W, b, W_prior,
        b_prior, expert_idx,
    )
    return out, br


def kernel(**inputs) -> np.ndarray:
    out, _ = run(inputs, STRATEGY)
    return out


# revision 4
# speedup vs baseline: 1.7801x; 1.0098x over previous
"""Trainium2 Bass kernel for nn_EnsembleLayer (MoE one-hot routing).

Reference math (per token i, expert e = expert_idx[i]):
    out[i] = W[e] @ logits[i] + b[e] + W_prior[e] @ prior_logits[i] + b_prior[e]

Strategy:
  * Host-side routing: sort tokens by expert (the reference wastefully
    computes all 16 experts per token; we compute only the routed one).
  * Expert-parallel over 8 NeuronCores: core c owns experts (2c, 2c+1),
    each padded to a static capacity of C tokens.
  * Per expert slot the device computes  V.T @ Z  where
        Z = [X_e.T ; Xp_e.T]          (K=2048, C tokens)
        V = [W[e].T ; W_prior[e].T]   (K=2048, 1024 out)
    i.e. posterior and prior fused into one K=2048 contraction, both
    accumulating in the same PSUM tile.
  * Bias (zero in this problem, but handled anyway) is added on host.
  * Tokens overflowing the static capacity (cannot happen for the fixed
    seed's counts, max 295 < C) fall back to exact numpy on host.

Precision strategy for the matmuls (PE TensorEngine):
  * "fp32r" : single-pass FP32R (1s+8e+11m, 1 cyc/row at N>=256). ~1.5e-4
              max scale-relative error.
  * "bf16x3": bf16 hi/lo split, 3 accumulating passes
              (hi@hi + hi@lo + lo@hi). ~5e-6 max scale-relative error.
  * "fp32"  : native fp32 matmul (4 cyc/row). ~2e-7.
"""

import sys

sys.path.insert(0, "/opt/trn_rl_repo")

import ml_dtypes
import numpy as np

import concourse.mybir as mybir
import concourse.tile as tile
from concourse import bacc
from concourse.bass_utils import run_bass_kernel_spmd

dt = mybir.dt

# ---- problem constants (hardcoded per contract) ----
B = 4096
IN_F = 1024
OUT_F = 1024
E = 16
N_CORES = 8
EXPERTS_PER_CORE = E // N_CORES  # 2
P = 128
K = 2 * IN_F  # 2048: posterior + prior fused contraction
KO = K // P  # 16 k-tiles
MO = OUT_F // P  # 8 m-tiles
C = 320  # static token capacity per expert (seed-0 max count is 295)

STRATEGY = "bf16x3"  # one of: "fp32r", "bf16x3", "fp32"


def _round_fp32r(x: np.ndarray) -> np.ndarray:
    """Round fp32 to the FP32R grid (1s+8e+11m, RN-even)."""
    u = np.ascontiguousarray(x).view(np.uint32)
    r = (u + np.uint32(0x7FF) + ((u >> np.uint32(12)) & np.uint32(1))) & np.uint32(
        0xFFFFF000
    )
    return r.view(np.float32)


def _split_bf16(x: np.ndarray):
    hi = x.astype(ml_dtypes.bfloat16)
    lo = (x - hi.astype(np.float32)).astype(ml_dtypes.bfloat16)
    return hi, lo


def build_nc(strategy: str = STRATEGY):
    """Build the SPMD per-core Bass program.

    Inputs per core (leading dim j = expert slot 0/1):
      zt  [2, P, KO, C]      tokens, k-tiled transposed   (fp32r / fp32)
      vt  [2, MO, P, KO, P]  weights, k&m-tiled           (fp32r / fp32)
      (bf16x3: zhi/zlo and vhi/vlo in bf16 instead)
    Output:
      yt  [2, MO, P, C] fp32   yt[j,m,mi,n] = out-feature m*P+mi of token n
    """
    nc = bacc.Bacc("TRN2", target_bir_lowering=False, debug=False)

    if strategy == "bf16x3":
        zhi = nc.dram_tensor("zhi", [2, P, KO, C], dt.bfloat16, kind="ExternalInput").ap()
        zlo = nc.dram_tensor("zlo", [2, P, KO, C], dt.bfloat16, kind="ExternalInput").ap()
        vhi = nc.dram_tensor("vhi", [2, MO, P, KO, P], dt.bfloat16, kind="ExternalInput").ap()
        vlo = nc.dram_tensor("vlo", [2, MO, P, KO, P], dt.bfloat16, kind="ExternalInput").ap()
    else:
        mm_dt = dt.float32r if strategy == "fp32r" else dt.float32
        zt = nc.dram_tensor("zt", [2, P, KO, C], mm_dt, kind="ExternalInput").ap()
        vt = nc.dram_tensor("vt", [2, MO, P, KO, P], mm_dt, kind="ExternalInput").ap()
    yt = nc.dram_tensor("yt", [2, MO, P, C], dt.float32, kind="ExternalOutput").ap()

    with tile.TileContext(nc) as tc:
        with (
            tc.tile_pool(name="zp", bufs=1) as zp,
            tc.tile_pool(name="wp", bufs=4) as wp,
            tc.tile_pool(name="ps", bufs=8, space="PSUM") as ps,
            tc.tile_pool(name="op", bufs=4) as op,
        ):
            if strategy == "bf16x3":
                z_hi = []
                z_lo = []
                for j in range(2):
                    th = zp.tile([P, KO, C], dt.bfloat16, tag=f"zhi{j}")
                    nc.sync.dma_start(th[:], zhi[j])
                    tl = zp.tile([P, KO, C], dt.bfloat16, tag=f"zlo{j}")
                    nc.sync.dma_start(tl[:], zlo[j])
                    z_hi.append(th)
                    z_lo.append(tl)
            else:
                z_tiles = []
                for j in range(2):
                    t = zp.tile([P, KO, C], zt.dtype, tag=f"z{j}")
                    nc.sync.dma_start(t[:], zt[j])
                    z_tiles.append(t)

            for j in range(2):
                for m in range(MO):
                    if strategy == "bf16x3":
                        wh = wp.tile([P, KO, P], dt.bfloat16, tag="wh")
                        nc.sync.dma_start(wh[:], vhi[j, m])
                        wl = wp.tile([P, KO, P], dt.bfloat16, tag="wl")
                        nc.sync.dma_start(wl[:], vlo[j, m])
                        # k-major with wh reused for (zhi, zlo) back-to-back
                        # so walrus can share the weight load
                        steps = []
                        for k in range(KO):
                            steps.append((wh, z_hi[j], k))
                            steps.append((wh, z_lo[j], k))
                        for k in range(KO):
                            steps.append((wl, z_hi[j], k))
                    else:
                        w = wp.tile([P, KO, P], vt.dtype, tag="w")
                        nc.sync.dma_start(w[:], vt[j, m])
                        steps = [(w, z_tiles[j], k) for k in range(KO)]

                    pt = ps.tile([P, C], dt.float32, tag="psum")
                    n_mm = len(steps)
                    for i, (wt_t, zt_t, k) in enumerate(steps):
                        nc.tensor.matmul(
                            pt[:],
                            wt_t[:, k, :],
                            zt_t[:, k, :],
                            start=(i == 0),
                            stop=(i == n_mm - 1),
                        )
                    o = op.tile([P, C], dt.float32, tag="out")
                    # explicit DVE: nc.any routes this to ScalarE (9x slower)
                    nc.vector.tensor_copy(out=o[:], in_=pt[:])
                    nc.sync.dma_start(yt[j, m], o[:])

    nc.compile()
    return nc


_NC_CACHE: dict = {}


def _get_nc(strategy: str):
    if strategy not in _NC_CACHE:
        _NC_CACHE[strategy] = build_nc(strategy)
    return _NC_CACHE[strategy]


def _prepare_in_maps(logits, prior_logits, W, W_prior, expert_idx, strategy):
    """Route tokens and build the 8 per-core input maps.

    Returns (in_maps, routed_idx, overflow) where routed_idx[e] is the array
    of token indices assigned to expert e's capacity slots (in slot order)
    and overflow is the list of (token_idx, expert) that did not fit.
    """
    in_maps = []
    routed_idx = []
    overflow = []
    for e in range(E):
        idx = np.nonzero(expert_idx == e)[0]
        if len(idx) > C:
            overflow.extend((int(i), e) for i in idx[C:])
            idx = idx[:C]
        routed_idx.append(idx)

    for c in range(N_CORES):
        zt_c = np.zeros((2, P, KO, C), np.float32)
        vt_c = np.empty((2, MO, P, KO, P), np.float32)
        for j in range(EXPERTS_PER_CORE):
            e = EXPERTS_PER_CORE * c + j
            idx = routed_idx[e]
            n_e = len(idx)
            # Z = [X_e.T ; Xp_e.T]  -> [K, C] -> tiled [P, KO, C]
            Z = np.zeros((K, C), np.float32)
            Z[:IN_F, :n_e] = logits[idx].T
            Z[IN_F:, :n_e] = prior_logits[idx].T
            zt_c[j] = Z.reshape(KO, P, C).transpose(1, 0, 2)
            # V = [W[e].T ; Wp[e].T] -> [K, OUT_F] -> tiled [MO, P, KO, P]
            V = np.concatenate([W[e].T, W_prior[e].T], axis=0)
            vt_c[j] = V.reshape(KO, P, MO, P).transpose(2, 1, 0, 3)

        if strategy == "bf16x3":
            zhi, zlo = _split_bf16(zt_c)
            vhi, vlo = _split_bf16(vt_c)
            in_maps.append({"zhi": zhi, "zlo": zlo, "vhi": vhi, "vlo": vlo})
        elif strategy == "fp32r":
            in_maps.append({"zt": _round_fp32r(zt_c), "vt": _round_fp32r(vt_c)})
        else:
            in_maps.append({"zt": zt_c, "vt": vt_c})
    return in_maps, routed_idx, overflow


def _gather_output(results, routed_idx, overflow, logits, prior_logits, W, b,
                   W_prior, b_prior, expert_idx):
    out = np.empty((B, OUT_F), np.float32)
    for c in range(N_CORES):
        yt = results[c]["yt"]  # [2, MO, P, C]
        for j in range(EXPERTS_PER_CORE):
            e = EXPERTS_PER_CORE * c + j
            idx = routed_idx[e]
            if len(idx) == 0:
                continue
            y = yt[j].reshape(OUT_F, C)  # [out, C]
            out[idx] = y[:, : len(idx)].T
    # bias (zero in this problem, added for faithfulness)
    bias = b + b_prior  # [E, OUT_F]
    if np.any(bias):
        out += bias[expert_idx]
    for i, e in overflow:
        out[i] = (
            W[e] @ logits[i]
            + b[e]
            + W_prior[e] @ prior_logits[i]
            + b_prior[e]
        )
    return out[:, None, :]


def run(inputs: dict, strategy: str = STRATEGY, trace: bool = False):
    """Run the kernel; returns (output, BassKernelResults)."""
    logits = np.asarray(inputs["logits"], np.float32)
    prior_logits = np.asarray(inputs["prior_logits"], np.float32)
    W = np.asarray(inputs["W"], np.float32)
    b = np.asarray(inputs["b"], np.float32)
    W_prior = np.asarray(inputs["W_prior"], np.float32)
    b_prior = np.asarray(inputs["b_prior"], np.float32)
    expert_idx = np.asarray(inputs["expert_idx"])

    nc = _get_nc(strategy)
    in_maps, routed_idx, overflow = _prepare_in_maps(
        logits, prior_logits, W, W_prior, expert_idx, strategy
    )
    br = run_bass_kernel_spmd(nc, in_maps, list(range(N_CORES)), trace=trace)
    out = _gather_output(
        br.results, routed_idx, overflow, logits, prior_logits, W, b, W_prior,
        b_prior, expert_idx,
    )
    return out, br


def kernel(**inputs) -> np.ndarray:
    out, _ = run(inputs, STRATEGY)
    return out
